# revision 16
# baseline (speedup 1.0000x reference)
"""InfoVAE loss kernel for Trainium2, data-parallel over batch on 8 NeuronCores.

Reference computation (see problem spec):
    recons_loss = mean((recons - x)^2)                    recons/x: [4096, 3, 64, 64]
    mmd  = km(pz,pz) + km(z,z) - 2*km(pz,z)               z/pz:     [4096, 128]
           where km(a,b) = mean_ij exp(-(|a_i-b_j|^2/D)/sigma), sigma = 2*D*z_var
    kld  = mean_n(-0.5 * sum_d(1 + lv - mu^2 - exp(lv)))
    loss = 5*recons_loss + 1.5*(1/N)*kld + 98.5/(N*(N-1))*mmd
    returns (loss, recons_loss, mmd, -kld)

Sharding: each core owns a 512-row block of the batch. The RBF kernel blocks are
computed as block-rows vs the full gathered z/prior_z (replicated, 2 MB each).
Per-core partial sums come back as small per-partition accumulator tiles; the
final (tiny) reduction is done on host in float64.

RBF assembly on device: arg_ij = a_i.b_j/32768 - |a_i|^2/65536 - |b_j|^2/65536.
 - a_i.b_j/32768 : PE matmul with the block lhsT pre-scaled by 2^-15 (exact).
 - -|b_j|^2/65536: a K=1 accumulating matmul (ones outer-product row term).
 - -|a_i|^2/65536: per-partition bias of the ACT Exp instruction.
ACT's fused accum_out gives the per-partition running sums for free.

All RBF-path matmul operands are float32r (TF32-like): 1 PE cycle/column vs
fp32's 4, and measured max rel err 1.6e-4 on HW dot products — far inside
this loss's tolerance. Operand tiles are rounded to f32r by their producer
(DVE copy / mul), which the BIR verifier requires.
"""

import numpy as np

N = 4096
D = 128
NCORES = 8
ROWS = N // NCORES            # 512 rows per core
IMG_F = 3 * 64 * 64           # 12288
P = 128
T_ROW = ROWS // P             # 4 row tiles per core
MSE_CHUNK = 2048
MSE_NCH = IMG_F // MSE_CHUNK  # 6
JG = 1024                     # psum group width for the rbf matmuls
NJG = N // JG                 # 4 j-groups
Z_VAR = 2.0
SIGMA = 2.0 * D * Z_VAR       # 512
INV_2S = 1.0 / (D * SIGMA / 2.0)   # 1/32768 (exact power of two)
INV_S = 1.0 / (D * SIGMA)          # 1/65536

_CACHE = {}


def _build():
    import concourse.bass as bass
    import concourse.tile as tile
    from concourse import bacc, mybir

    f32 = mybir.dt.float32
    f32r = mybir.dt.float32r
    AF = mybir.ActivationFunctionType
    ALU = mybir.AluOpType
    AX = mybir.AxisListType

    nc = bacc.Bacc("TRN2", target_bir_lowering=False, debug=False,
                   num_devices=NCORES)

    r_blk = nc.dram_tensor("r_blk", [ROWS, IMG_F], f32, kind="ExternalInput").ap()
    x_blk = nc.dram_tensor("x_blk", [ROWS, IMG_F], f32, kind="ExternalInput").ap()
    z_full = nc.dram_tensor("z_full", [N, D], f32, kind="ExternalInput").ap()
    pz_full = nc.dram_tensor("pz_full", [N, D], f32, kind="ExternalInput").ap()
    z_blk = nc.dram_tensor("z_blk", [ROWS, D], f32, kind="ExternalInput").ap()
    pz_blk = nc.dram_tensor("pz_blk", [ROWS, D], f32, kind="ExternalInput").ap()
    mu_blk = nc.dram_tensor("mu_blk", [ROWS, D], f32, kind="ExternalInput").ap()
    lv_blk = nc.dram_tensor("lv_blk", [ROWS, D], f32, kind="ExternalInput").ap()
    ident = nc.dram_tensor("ident", [P, P], f32, kind="ExternalInput").ap()

    NMSE = T_ROW * MSE_NCH            # 24 chunks
    NMSE_COLS = NMSE + 3              # last chunk split into 4 strip columns
    NMMD = 3 * T_ROW * NJG            # 48 accum columns
    mse_out = nc.dram_tensor("mse_acc", [P, NMSE_COLS], f32, kind="ExternalOutput").ap()
    mmd_out = nc.dram_tensor("mmd_acc", [P, NMMD], f32, kind="ExternalOutput").ap()
    kld_out = nc.dram_tensor("kld_acc", [P, 4], f32, kind="ExternalOutput").ap()

    with tile.TileContext(nc) as tc:
        with (
            tc.tile_pool(name="consts", bufs=1) as consts,
            tc.tile_pool(name="nat", bufs=1) as nat,
            tc.tile_pool(name="stream", bufs=4) as stream,
            tc.tile_pool(name="dpool", bufs=2) as dpool,
            tc.tile_pool(name="tstage", bufs=2) as tstage,
            tc.tile_pool(name="scratch", bufs=2) as scratch,
            tc.tile_pool(name="acc", bufs=1) as accp,
            tc.tile_pool(name="psmm", bufs=3, space="PSUM") as psmm,
            tc.tile_pool(name="pstr", bufs=2, space="PSUM") as pstr,
        ):
            # ---- constants / small setup ----
            ident_sb = consts.tile([P, P], f32)
            nc.sync.dma_start(out=ident_sb[:], in_=ident)
            ones_row_f = consts.tile([1, P], f32)
            nc.vector.memset(ones_row_f[:], 1.0)
            ones_row = consts.tile([1, P], f32r)   # memset can't emit f32r
            nc.scalar.activation(out=ones_row[:], in_=ones_row_f[:], func=AF.Copy)
            negs_col = consts.tile([P, 1], f32)       # -1/65536 column for norm matmuls
            nc.vector.memset(negs_col[:], -INV_S)

            # accumulators
            mse_cols = accp.tile([P, NMSE_COLS], f32)
            mmd_cols = accp.tile([P, NMMD], f32)
            kld_cols = accp.tile([P, 4], f32)
            nc.vector.memset(kld_cols[:, 3:4], 0.0)

            zv = z_full.rearrange("(t p) d -> p t d", p=P)
            pv = pz_full.rearrange("(t p) d -> p t d", p=P)

            # block rows natural (for bias norms + block transpose)
            zb_nat = nat.tile([P, T_ROW, D], f32)
            pb_nat = nat.tile([P, T_ROW, D], f32)
            nc.sync.dma_start(out=zb_nat[:], in_=z_blk.rearrange("(t p) d -> p t d", p=P))
            nc.sync.dma_start(out=pb_nat[:], in_=pz_blk.rearrange("(t p) d -> p t d", p=P))

            rv = r_blk.rearrange("(t p) f -> p t f", p=P)
            xv = x_blk.rearrange("(t p) f -> p t f", p=P)

            def emit_mse(k):
                # MSE stream unit: DMA r/x chunk, DVE subtract, ACT square+accum.
                t, c = divmod(k, MSE_NCH)
                rt = stream.tile([P, MSE_CHUNK], f32, tag="rt")
                xt = stream.tile([P, MSE_CHUNK], f32, tag="xt")
                # split each chunk load 4 ways so multiple DMA queues fill one
                # buffer concurrently (per-queue bw is ~27 GiB/s)
                w = MSE_CHUNK // 4
                for h in range(4):
                    lo = c * MSE_CHUNK + h * w
                    nc.sync.dma_start(out=rt[:, h * w:(h + 1) * w],
                                      in_=rv[:, t, lo:lo + w])
                    nc.sync.dma_start(out=xt[:, h * w:(h + 1) * w],
                                      in_=xv[:, t, lo:lo + w])
                dt = dpool.tile([P, MSE_CHUNK], f32)
                sc = scratch.tile([P, MSE_CHUNK], f32, tag="msq")
                if k == NMSE - 1:
                    # last chunk: 4 column strips so the post-DMA tail chain
                    # pipelines sub(DVE) against square(ACT) per strip.
                    ws = MSE_CHUNK // 4
                    for s in range(4):
                        col = k if s == 0 else NMSE + s - 1
                        sl = slice(s * ws, (s + 1) * ws)
                        nc.vector.tensor_sub(dt[:, sl], rt[:, sl], xt[:, sl])
                        nc.scalar.activation(out=sc[:, sl], in_=dt[:, sl],
                                             func=AF.Square,
                                             accum_out=mse_cols[:, col:col + 1])
                    return
                nc.vector.tensor_sub(dt[:], rt[:], xt[:])
                nc.scalar.activation(out=sc[:], in_=dt[:], func=AF.Square,
                                     accum_out=mse_cols[:, k:k + 1])

            # ---- transpose z/pz to [d, j] layout via PE (staged loads) ----
            # PSUM->SBUF copies run on ACT (Copy) so the DVE queue stays free
            # for the MSE subtracts; MSE stream units are threaded between
            # staging groups to keep the DMA bus saturated from t=0.
            zT = consts.tile([P, N], f32r)
            pzT = consts.tile([P, N], f32r)
            mse_next = 0

            def stage_transpose(view, dst):
                nonlocal mse_next
                for g in range(4):                # stage 8 row-tiles (1 MB) at a time
                    st = tstage.tile([P, 8, D], f32, tag="tst")
                    nc.sync.dma_start(out=st[:, 0:4, :], in_=view[:, g * 8:g * 8 + 4, :])
                    nc.sync.dma_start(out=st[:, 4:8, :], in_=view[:, g * 8 + 4:g * 8 + 8, :])
                    for gg in range(2):           # 4 transposes per psum tile
                        tp = pstr.tile([P, 512], f32, tag="tr")
                        for k in range(4):
                            nc.tensor.transpose(tp[:, k * P:(k + 1) * P],
                                                st[:, gg * 4 + k, :], ident_sb[:])
                        col = (g * 8 + gg * 4) * P
                        nc.vector.tensor_copy(dst[:, col:col + 512], tp[:])
                    if g % 2 == 1:
                        emit_mse(mse_next)
                        mse_next += 1

            emit_mse(mse_next); mse_next += 1
            # pz first: the first MMD pair k(pz,pz) needs pzT/nn_pz earliest
            stage_transpose(pv, pzT)
            stage_transpose(zv, zT)

            # block transposed & pre-scaled by 1/32768 (exact pow2, via ACT)
            zbTs = consts.tile([P, ROWS], f32r)
            pbTs = consts.tile([P, ROWS], f32r)
            for (src, dst) in ((pb_nat, pbTs), (zb_nat, zbTs)):
                tp = pstr.tile([P, 512], f32, tag="tr")
                for t in range(T_ROW):
                    nc.tensor.transpose(tp[:, t * P:(t + 1) * P], src[:, t, :],
                                        ident_sb[:])
                nc.scalar.activation(out=dst[:], in_=tp[:], func=AF.Copy,
                                     scale=INV_2S)

            emit_mse(mse_next); mse_next += 1

            # ---- column norm rows: negnorm[j] = -|b_j|^2/65536, laid [1, N] ----
            # squares on the (otherwise idle) Pool engine, norm matmul fp32,
            # PSUM->SBUF result copy on ACT with f32r rounding.
            nn_z = consts.tile([1, N], f32r)
            nn_pz = consts.tile([1, N], f32r)
            for (srcT, dst) in ((pzT, nn_pz), (zT, nn_z)):
                for c in range(N // 512):
                    sq = scratch.tile([P, 512], f32, tag="sq")
                    nc.gpsimd.tensor_mul(sq[:], srcT[:, c * 512:(c + 1) * 512].bitcast(f32),
                                         srcT[:, c * 512:(c + 1) * 512].bitcast(f32))
                    npm = pstr.tile([P, 512], f32, tag="tr")
                    nc.tensor.matmul(npm[0:1, :], lhsT=negs_col[:], rhs=sq[:],
                                     start=True, stop=True)
                    nc.scalar.activation(out=dst[0:1, c * 512:(c + 1) * 512],
                                         in_=npm[0:1, :], func=AF.Copy)
                emit_mse(mse_next); mse_next += 1

            # ---- row-bias tiles: bias_a[:, t] = -|a_i|^2/65536 for block rows ----
            bias_z = consts.tile([P, T_ROW], f32)
            bias_pz = consts.tile([P, T_ROW], f32)
            for (src, dst) in ((pb_nat, bias_pz), (zb_nat, bias_z)):
                for t in range(T_ROW):
                    sq2 = scratch.tile([P, D], f32, tag="sq2")
                    # Square(x/256) = x^2/65536 (scale is an exact pow2)
                    nc.scalar.activation(out=sq2[:], in_=src[:, t, :],
                                         func=AF.Square, scale=1.0 / 256.0,
                                         accum_out=dst[:, t:t + 1])
                nc.gpsimd.tensor_scalar_mul(dst[:], dst[:], -1.0)

            # ---- KLD block terms ----
            mu_t = nat.tile([P, T_ROW, D], f32)
            lv_t = nat.tile([P, T_ROW, D], f32)
            nc.sync.dma_start(out=mu_t[:], in_=mu_blk.rearrange("(t p) d -> p t d", p=P))
            nc.sync.dma_start(out=lv_t[:], in_=lv_blk.rearrange("(t p) d -> p t d", p=P))
            ksc = scratch.tile([P, T_ROW, D], f32, tag="ksc")
            nc.vector.tensor_reduce(kld_cols[:, 0:1], lv_t[:], axis=AX.XY,
                                    op=ALU.add)
            nc.scalar.activation(out=ksc[:], in_=mu_t[:], func=AF.Square,
                                 accum_out=kld_cols[:, 1:2])
            ksc2 = scratch.tile([P, T_ROW, D], f32, tag="ksc")
            nc.scalar.activation(out=ksc2[:], in_=lv_t[:], func=AF.Exp,
                                 accum_out=kld_cols[:, 2:3])

            # ---- interleaved main loops: MMD rbf blocks + MSE stream ----
            pairs = [(pbTs, pzT, nn_pz, bias_pz),   # k(pz, pz)
                     (zbTs, zT, nn_z, bias_z),      # k(z, z)
                     (pbTs, zT, nn_z, bias_pz)]     # k(pz, z)

            def emit_mmd(k):
                pi, rem = divmod(k, T_ROW * NJG)
                t, jg = divmod(rem, NJG)
                aTs, bT, nn_b, bias_a = pairs[pi]
                ps = psmm.tile([P, JG], f32)
                for jc in range(JG // 512):
                    j = jg * (JG // 512) + jc
                    nc.tensor.matmul(ps[:, jc * 512:(jc + 1) * 512],
                                     lhsT=aTs[:, t * P:(t + 1) * P],
                                     rhs=bT[:, j * 512:(j + 1) * 512],
                                     start=True, stop=False)
                    nc.tensor.matmul(ps[:, jc * 512:(jc + 1) * 512],
                                     lhsT=ones_row[:], rhs=nn_b[0:1, j * 512:(j + 1) * 512],
                                     start=False, stop=True)
                sc = scratch.tile([P, JG], f32, tag="esc")
                nc.scalar.activation(out=sc[:], in_=ps[:], func=AF.Exp,
                                     bias=bias_a[:, t:t + 1], scale=1.0,
                                     accum_out=mmd_cols[:, k:k + 1])

            # Front-load MMD 3:1 against the MSE stream: all MMD (PE/ACT) work
            # finishes well before the DMA stream ends, so the kernel tail is
            # just the last chunk's subtract+square.
            mmd_next = 0
            while mmd_next < NMMD or mse_next < NMSE:
                for _ in range(3):
                    if mmd_next < NMMD:
                        emit_mmd(mmd_next)
                        mmd_next += 1
                if mse_next < NMSE:
                    emit_mse(mse_next)
                    mse_next += 1

            # ---- write partials out ----
            nc.sync.dma_start(out=mse_out, in_=mse_cols[:])
            nc.sync.dma_start(out=mmd_out, in_=mmd_cols[:])
            nc.sync.dma_start(out=kld_out, in_=kld_cols[:])

    nc.compile()
    return nc


def get_nc():
    if "nc" not in _CACHE:
        _CACHE["nc"] = _build()
    return _CACHE["nc"]


def make_in_maps(recons, x, z, mu, log_var, prior_z):
    r2 = np.ascontiguousarray(recons, dtype=np.float32).reshape(N, IMG_F)
    x2 = np.ascontiguousarray(x, dtype=np.float32).reshape(N, IMG_F)
    z = np.ascontiguousarray(z, dtype=np.float32)
    pz = np.ascontiguousarray(prior_z, dtype=np.float32)
    mu = np.ascontiguousarray(mu, dtype=np.float32)
    lv = np.ascontiguousarray(log_var, dtype=np.float32)
    ident = np.eye(P, dtype=np.float32)
    maps = []
    for c in range(NCORES):
        s = slice(c * ROWS, (c + 1) * ROWS)
        maps.append({
            "r_blk": r2[s], "x_blk": x2[s],
            "z_full": z, "pz_full": pz,
            "z_blk": z[s], "pz_blk": pz[s],
            "mu_blk": mu[s], "lv_blk": lv[s],
            "ident": ident,
        })
    return maps


def combine(results):
    mse_sum = 0.0
    s_pp = s_zz = s_pz = 0.0
    kld_total = 0.0
    per_pair = T_ROW * NJG
    for res in results:
        mse_sum += np.float64(res["mse_acc"]).sum()
        m = np.float64(res["mmd_acc"])
        s_pp += m[:, 0:per_pair].sum()
        s_zz += m[:, per_pair:2 * per_pair].sum()
        s_pz += m[:, 2 * per_pair:3 * per_pair].sum()
        k = np.float64(res["kld_acc"])
        kld_total += ROWS * D + k[:, 0].sum() - k[:, 1].sum() - k[:, 2].sum()

    recons_loss = mse_sum / (N * IMG_F)
    mmd = (s_pp + s_zz - 2.0 * s_pz) / (float(N) * float(N))
    kld = -0.5 * kld_total / N
    beta, alpha, reg_w = 5.0, -0.5, 100.0
    loss = (beta * recons_loss
            + (1.0 - alpha) * (1.0 / N) * kld
            + (alpha + reg_w - 1.0) / (float(N) * (N - 1)) * mmd)
    return (np.float32(loss), np.float32(recons_loss),
            np.float32(mmd), np.float32(-kld))


def run(recons, x, z, mu, log_var, prior_z, trace=False):
    from concourse.bass_utils import run_bass_kernel_spmd
    nc = get_nc()
    in_maps = make_in_maps(recons, x, z, mu, log_var, prior_z)
    res = run_bass_kernel_spmd(nc, in_maps, list(range(NCORES)), trace=trace)
    return res


def kernel(recons, x, z, mu, log_var, prior_z):
    res = run(recons, x, z, mu, log_var, prior_z)
    return combine(res.results)



# revision 17
# speedup vs baseline: 1.0001x; 1.0001x over previous
"""InfoVAE loss kernel for Trainium2, data-parallel over batch on 8 NeuronCores.

Reference computation (see problem spec):
    recons_loss = mean((recons - x)^2)                    recons/x: [4096, 3, 64, 64]
    mmd  = km(pz,pz) + km(z,z) - 2*km(pz,z)               z/pz:     [4096, 128]
           where km(a,b) = mean_ij exp(-(|a_i-b_j|^2/D)/sigma), sigma = 2*D*z_var
    kld  = mean_n(-0.5 * sum_d(1 + lv - mu^2 - exp(lv)))
    loss = 5*recons_loss + 1.5*(1/N)*kld + 98.5/(N*(N-1))*mmd
    returns (loss, recons_loss, mmd, -kld)

Sharding: each core owns a 512-row block of the batch. The RBF kernel blocks are
computed as block-rows vs the full gathered z/prior_z (replicated, 2 MB each).
Per-core partial sums come back as small per-partition accumulator tiles; the
final (tiny) reduction is done on host in float64.

RBF assembly on device: arg_ij = a_i.b_j/32768 - |a_i|^2/65536 - |b_j|^2/65536.
 - a_i.b_j/32768 : PE matmul with the block lhsT pre-scaled by 2^-15 (exact).
 - -|b_j|^2/65536: a K=1 accumulating matmul (ones outer-product row term).
 - -|a_i|^2/65536: per-partition bias of the ACT Exp instruction.
ACT's fused accum_out gives the per-partition running sums for free.

All RBF-path matmul operands are float32r (TF32-like): 1 PE cycle/column vs
fp32's 4, and measured max rel err 1.6e-4 on HW dot products — far inside
this loss's tolerance. Operand tiles are rounded to f32r by their producer
(DVE copy / mul), which the BIR verifier requires.
"""

import numpy as np

N = 4096
D = 128
NCORES = 8
ROWS = N // NCORES            # 512 rows per core
IMG_F = 3 * 64 * 64           # 12288
P = 128
T_ROW = ROWS // P             # 4 row tiles per core
MSE_CHUNK = 2048
MSE_NCH = IMG_F // MSE_CHUNK  # 6
JG = 1024                     # psum group width for the rbf matmuls
NJG = N // JG                 # 4 j-groups
Z_VAR = 2.0
SIGMA = 2.0 * D * Z_VAR       # 512
INV_2S = 1.0 / (D * SIGMA / 2.0)   # 1/32768 (exact power of two)
INV_S = 1.0 / (D * SIGMA)          # 1/65536

_CACHE = {}


def _build():
    import concourse.bass as bass
    import concourse.tile as tile
    from concourse import bacc, mybir

    f32 = mybir.dt.float32
    f32r = mybir.dt.float32r
    AF = mybir.ActivationFunctionType
    ALU = mybir.AluOpType
    AX = mybir.AxisListType

    nc = bacc.Bacc("TRN2", target_bir_lowering=False, debug=False,
                   num_devices=NCORES)

    r_blk = nc.dram_tensor("r_blk", [ROWS, IMG_F], f32, kind="ExternalInput").ap()
    x_blk = nc.dram_tensor("x_blk", [ROWS, IMG_F], f32, kind="ExternalInput").ap()
    z_full = nc.dram_tensor("z_full", [N, D], f32, kind="ExternalInput").ap()
    pz_full = nc.dram_tensor("pz_full", [N, D], f32, kind="ExternalInput").ap()
    z_blk = nc.dram_tensor("z_blk", [ROWS, D], f32, kind="ExternalInput").ap()
    pz_blk = nc.dram_tensor("pz_blk", [ROWS, D], f32, kind="ExternalInput").ap()
    mu_blk = nc.dram_tensor("mu_blk", [ROWS, D], f32, kind="ExternalInput").ap()
    lv_blk = nc.dram_tensor("lv_blk", [ROWS, D], f32, kind="ExternalInput").ap()
    ident = nc.dram_tensor("ident", [P, P], f32, kind="ExternalInput").ap()

    NMSE = T_ROW * MSE_NCH            # 24 chunks
    NMSE_COLS = NMSE + 3              # last chunk split into 4 strip columns
    NMMD = 3 * T_ROW * NJG            # 48 accum columns
    mse_out = nc.dram_tensor("mse_acc", [P, NMSE_COLS], f32, kind="ExternalOutput").ap()
    mmd_out = nc.dram_tensor("mmd_acc", [P, NMMD], f32, kind="ExternalOutput").ap()
    kld_out = nc.dram_tensor("kld_acc", [P, 4], f32, kind="ExternalOutput").ap()

    with tile.TileContext(nc) as tc:
        with (
            tc.tile_pool(name="consts", bufs=1) as consts,
            tc.tile_pool(name="nat", bufs=1) as nat,
            tc.tile_pool(name="stream", bufs=4) as stream,
            tc.tile_pool(name="dpool", bufs=2) as dpool,
            tc.tile_pool(name="tstage", bufs=2) as tstage,
            tc.tile_pool(name="scratch", bufs=2) as scratch,
            tc.tile_pool(name="acc", bufs=1) as accp,
            tc.tile_pool(name="psmm", bufs=3, space="PSUM") as psmm,
            tc.tile_pool(name="pstr", bufs=2, space="PSUM") as pstr,
        ):
            # ---- constants / small setup ----
            ident_sb = consts.tile([P, P], f32)
            nc.sync.dma_start(out=ident_sb[:], in_=ident)
            ones_row_f = consts.tile([1, P], f32)
            nc.vector.memset(ones_row_f[:], 1.0)
            ones_row = consts.tile([1, P], f32r)   # memset can't emit f32r
            nc.scalar.activation(out=ones_row[:], in_=ones_row_f[:], func=AF.Copy)
            negs_col = consts.tile([P, 1], f32)       # -1/65536 column for norm matmuls
            nc.vector.memset(negs_col[:], -INV_S)

            # accumulators
            mse_cols = accp.tile([P, NMSE_COLS], f32)
            mmd_cols = accp.tile([P, NMMD], f32)
            kld_cols = accp.tile([P, 4], f32)
            nc.vector.memset(kld_cols[:, 3:4], 0.0)

            zv = z_full.rearrange("(t p) d -> p t d", p=P)
            pv = pz_full.rearrange("(t p) d -> p t d", p=P)

            # block rows natural (for bias norms + block transpose)
            zb_nat = nat.tile([P, T_ROW, D], f32)
            pb_nat = nat.tile([P, T_ROW, D], f32)
            nc.sync.dma_start(out=zb_nat[:], in_=z_blk.rearrange("(t p) d -> p t d", p=P))
            nc.sync.dma_start(out=pb_nat[:], in_=pz_blk.rearrange("(t p) d -> p t d", p=P))

            rv = r_blk.rearrange("(t p) f -> p t f", p=P)
            xv = x_blk.rearrange("(t p) f -> p t f", p=P)

            def emit_mse(k):
                # MSE stream unit: DMA r/x chunk, DVE subtract, ACT square+accum.
                t, c = divmod(k, MSE_NCH)
                rt = stream.tile([P, MSE_CHUNK], f32, tag="rt")
                xt = stream.tile([P, MSE_CHUNK], f32, tag="xt")
                # split each chunk load 4 ways so multiple DMA queues fill one
                # buffer concurrently (per-queue bw is ~27 GiB/s)
                w = MSE_CHUNK // 4
                for h in range(4):
                    lo = c * MSE_CHUNK + h * w
                    nc.sync.dma_start(out=rt[:, h * w:(h + 1) * w],
                                      in_=rv[:, t, lo:lo + w])
                    nc.sync.dma_start(out=xt[:, h * w:(h + 1) * w],
                                      in_=xv[:, t, lo:lo + w])
                dt = dpool.tile([P, MSE_CHUNK], f32)
                sc = scratch.tile([P, MSE_CHUNK], f32, tag="msq")
                if k == NMSE - 1:
                    # last chunk: 4 column strips so the post-DMA tail chain
                    # pipelines sub(DVE) against square(ACT) per strip.
                    ws = MSE_CHUNK // 4
                    for s in range(4):
                        col = k if s == 0 else NMSE + s - 1
                        sl = slice(s * ws, (s + 1) * ws)
                        nc.vector.tensor_sub(dt[:, sl], rt[:, sl], xt[:, sl])
                        nc.scalar.activation(out=sc[:, sl], in_=dt[:, sl],
                                             func=AF.Square,
                                             accum_out=mse_cols[:, col:col + 1])
                    return
                nc.vector.tensor_sub(dt[:], rt[:], xt[:])
                nc.scalar.activation(out=sc[:], in_=dt[:], func=AF.Square,
                                     accum_out=mse_cols[:, k:k + 1])

            # ---- transpose z/pz to [d, j] layout via PE (staged loads) ----
            # PSUM->SBUF copies run on ACT (Copy) so the DVE queue stays free
            # for the MSE subtracts; MSE stream units are threaded between
            # staging groups to keep the DMA bus saturated from t=0.
            zT = consts.tile([P, N], f32r)
            pzT = consts.tile([P, N], f32r)
            mse_next = 0

            def stage_transpose(view, dst):
                nonlocal mse_next
                for g in range(4):                # stage 8 row-tiles (1 MB) at a time
                    st = tstage.tile([P, 8, D], f32, tag="tst")
                    nc.sync.dma_start(out=st[:, 0:4, :], in_=view[:, g * 8:g * 8 + 4, :])
                    nc.sync.dma_start(out=st[:, 4:8, :], in_=view[:, g * 8 + 4:g * 8 + 8, :])
                    for gg in range(2):           # 4 transposes per psum tile
                        tp = pstr.tile([P, 512], f32, tag="tr")
                        for k in range(4):
                            nc.tensor.transpose(tp[:, k * P:(k + 1) * P],
                                                st[:, gg * 4 + k, :], ident_sb[:])
                        col = (g * 8 + gg * 4) * P
                        nc.vector.tensor_copy(dst[:, col:col + 512], tp[:])
                    if g % 2 == 1:
                        emit_mse(mse_next)
                        mse_next += 1

            emit_mse(mse_next); mse_next += 1
            # pz first: the first MMD pair k(pz,pz) needs pzT/nn_pz earliest
            stage_transpose(pv, pzT)
            stage_transpose(zv, zT)

            # block transposed & pre-scaled by 1/32768 (exact pow2, via ACT)
            zbTs = consts.tile([P, ROWS], f32r)
            pbTs = consts.tile([P, ROWS], f32r)
            for (src, dst) in ((pb_nat, pbTs), (zb_nat, zbTs)):
                tp = pstr.tile([P, 512], f32, tag="tr")
                for t in range(T_ROW):
                    nc.tensor.transpose(tp[:, t * P:(t + 1) * P], src[:, t, :],
                                        ident_sb[:])
                nc.scalar.activation(out=dst[:], in_=tp[:], func=AF.Copy,
                                     scale=INV_2S)

            emit_mse(mse_next); mse_next += 1

            # ---- column norm rows: negnorm[j] = -|b_j|^2/65536, laid [1, N] ----
            # squares on the (otherwise idle) Pool engine, norm matmul fp32,
            # PSUM->SBUF result copy on ACT with f32r rounding.
            nn_z = consts.tile([1, N], f32r)
            nn_pz = consts.tile([1, N], f32r)
            for (srcT, dst) in ((pzT, nn_pz), (zT, nn_z)):
                for c in range(N // 512):
                    sq = scratch.tile([P, 512], f32, tag="sq")
                    nc.gpsimd.tensor_mul(sq[:], srcT[:, c * 512:(c + 1) * 512].bitcast(f32),
                                         srcT[:, c * 512:(c + 1) * 512].bitcast(f32))
                    npm = pstr.tile([P, 512], f32, tag="tr")
                    nc.tensor.matmul(npm[0:1, :], lhsT=negs_col[:], rhs=sq[:],
                                     start=True, stop=True)
                    nc.vector.tensor_copy(dst[0:1, c * 512:(c + 1) * 512],
                                          npm[0:1, :])
                emit_mse(mse_next); mse_next += 1

            # ---- row-bias tiles: bias_a[:, t] = -|a_i|^2/65536 for block rows ----
            bias_z = consts.tile([P, T_ROW], f32)
            bias_pz = consts.tile([P, T_ROW], f32)
            for (src, dst) in ((pb_nat, bias_pz), (zb_nat, bias_z)):
                for t in range(T_ROW):
                    sq2 = scratch.tile([P, D], f32, tag="sq2")
                    # Square(x/256) = x^2/65536 (scale is an exact pow2)
                    nc.scalar.activation(out=sq2[:], in_=src[:, t, :],
                                         func=AF.Square, scale=1.0 / 256.0,
                                         accum_out=dst[:, t:t + 1])
                nc.gpsimd.tensor_scalar_mul(dst[:], dst[:], -1.0)

            # ---- KLD block terms ----
            mu_t = nat.tile([P, T_ROW, D], f32)
            lv_t = nat.tile([P, T_ROW, D], f32)
            nc.sync.dma_start(out=mu_t[:], in_=mu_blk.rearrange("(t p) d -> p t d", p=P))
            nc.sync.dma_start(out=lv_t[:], in_=lv_blk.rearrange("(t p) d -> p t d", p=P))
            ksc = scratch.tile([P, T_ROW, D], f32, tag="ksc")
            nc.vector.tensor_reduce(kld_cols[:, 0:1], lv_t[:], axis=AX.XY,
                                    op=ALU.add)
            nc.scalar.activation(out=ksc[:], in_=mu_t[:], func=AF.Square,
                                 accum_out=kld_cols[:, 1:2])
            ksc2 = scratch.tile([P, T_ROW, D], f32, tag="ksc")
            nc.scalar.activation(out=ksc2[:], in_=lv_t[:], func=AF.Exp,
                                 accum_out=kld_cols[:, 2:3])

            # ---- interleaved main loops: MMD rbf blocks + MSE stream ----
            pairs = [(pbTs, pzT, nn_pz, bias_pz),   # k(pz, pz)
                     (zbTs, zT, nn_z, bias_z),      # k(z, z)
                     (pbTs, zT, nn_z, bias_pz)]     # k(pz, z)

            def emit_mmd(k):
                pi, rem = divmod(k, T_ROW * NJG)
                t, jg = divmod(rem, NJG)
                aTs, bT, nn_b, bias_a = pairs[pi]
                ps = psmm.tile([P, JG], f32)
                for jc in range(JG // 512):
                    j = jg * (JG // 512) + jc
                    nc.tensor.matmul(ps[:, jc * 512:(jc + 1) * 512],
                                     lhsT=aTs[:, t * P:(t + 1) * P],
                                     rhs=bT[:, j * 512:(j + 1) * 512],
                                     start=True, stop=False)
                    nc.tensor.matmul(ps[:, jc * 512:(jc + 1) * 512],
                                     lhsT=ones_row[:], rhs=nn_b[0:1, j * 512:(j + 1) * 512],
                                     start=False, stop=True)
                sc = scratch.tile([P, JG], f32, tag="esc")
                nc.scalar.activation(out=sc[:], in_=ps[:], func=AF.Exp,
                                     bias=bias_a[:, t:t + 1], scale=1.0,
                                     accum_out=mmd_cols[:, k:k + 1])

            # Front-load MMD 3:1 against the MSE stream: all MMD (PE/ACT) work
            # finishes well before the DMA stream ends, so the kernel tail is
            # just the last chunk's subtract+square.
            mmd_next = 0
            while mmd_next < NMMD or mse_next < NMSE:
                for _ in range(3):
                    if mmd_next < NMMD:
                        emit_mmd(mmd_next)
                        mmd_next += 1
                if mse_next < NMSE:
                    emit_mse(mse_next)
                    mse_next += 1

            # ---- write partials out ----
            nc.sync.dma_start(out=mse_out, in_=mse_cols[:])
            nc.sync.dma_start(out=mmd_out, in_=mmd_cols[:])
            nc.sync.dma_start(out=kld_out, in_=kld_cols[:])

    nc.compile()
    return nc


def get_nc():
    if "nc" not in _CACHE:
        _CACHE["nc"] = _build()
    return _CACHE["nc"]


def make_in_maps(recons, x, z, mu, log_var, prior_z):
    r2 = np.ascontiguousarray(recons, dtype=np.float32).reshape(N, IMG_F)
    x2 = np.ascontiguousarray(x, dtype=np.float32).reshape(N, IMG_F)
    z = np.ascontiguousarray(z, dtype=np.float32)
    pz = np.ascontiguousarray(prior_z, dtype=np.float32)
    mu = np.ascontiguousarray(mu, dtype=np.float32)
    lv = np.ascontiguousarray(log_var, dtype=np.float32)
    ident = np.eye(P, dtype=np.float32)
    maps = []
    for c in range(NCORES):
        s = slice(c * ROWS, (c + 1) * ROWS)
        maps.append({
            "r_blk": r2[s], "x_blk": x2[s],
            "z_full": z, "pz_full": pz,
            "z_blk": z[s], "pz_blk": pz[s],
            "mu_blk": mu[s], "lv_blk": lv[s],
            "ident": ident,
        })
    return maps


def combine(results):
    mse_sum = 0.0
    s_pp = s_zz = s_pz = 0.0
    kld_total = 0.0
    per_pair = T_ROW * NJG
    for res in results:
        mse_sum += np.float64(res["mse_acc"]).sum()
        m = np.float64(res["mmd_acc"])
        s_pp += m[:, 0:per_pair].sum()
        s_zz += m[:, per_pair:2 * per_pair].sum()
        s_pz += m[:, 2 * per_pair:3 * per_pair].sum()
        k = np.float64(res["kld_acc"])
        kld_total += ROWS * D + k[:, 0].sum() - k[:, 1].sum() - k[:, 2].sum()

    recons_loss = mse_sum / (N * IMG_F)
    mmd = (s_pp + s_zz - 2.0 * s_pz) / (float(N) * float(N))
    kld = -0.5 * kld_total / N
    beta, alpha, reg_w = 5.0, -0.5, 100.0
    loss = (beta * recons_loss
            + (1.0 - alpha) * (1.0 / N) * kld
            + (alpha + reg_w - 1.0) / (float(N) * (N - 1)) * mmd)
    return (np.float32(loss), np.float32(recons_loss),
            np.float32(mmd), np.float32(-kld))


def run(recons, x, z, mu, log_var, prior_z, trace=False):
    from concourse.bass_utils import run_bass_kernel_spmd
    nc = get_nc()
    in_maps = make_in_maps(recons, x, z, mu, log_var, prior_z)
    res = run_bass_kernel_spmd(nc, in_maps, list(range(NCORES)), trace=trace)
    return res


def kernel(recons, x, z, mu, log_var, prior_z):
    res = run(recons, x, z, mu, log_var, prior_z)
    return combine(res.results)



# revision 36
# speedup vs baseline: 1.0424x; 1.0423x over previous
"""InfoVAE loss kernel for Trainium2, data-parallel over batch on 8 NeuronCores.

Reference computation (see problem spec):
    recons_loss = mean((recons - x)^2)                    recons/x: [4096, 3, 64, 64]
    mmd  = km(pz,pz) + km(z,z) - 2*km(pz,z)               z/pz:     [4096, 128]
           where km(a,b) = mean_ij exp(-(|a_i-b_j|^2/D)/sigma), sigma = 2*D*z_var
    kld  = mean_n(-0.5 * sum_d(1 + lv - mu^2 - exp(lv)))
    loss = 5*recons_loss + 1.5*(1/N)*kld + 98.5/(N*(N-1))*mmd
    returns (loss, recons_loss, mmd, -kld)

Sharding:
 - MSE/KLD: each core owns a 512-row block of the batch (recons/x/mu/log_var
   sliced by the host).
 - MMD: the three 4096x4096 RBF-sum matrices (pz*pz, z*z, pz*z) are tiled as
   512-row strips x 2048-column halves. Each core receives SIX 512-row strips
   (a_nat, 1.5 MB) and ONE 2048-row column-half (b_nat, 1.0 MB) and computes
   the six strip-units against that single column-half -- 2.5 MB of HBM per
   core instead of replicating both full tensors (4.2 MB). Cross terms use
   k(a,b) = k(b,a) to flip half the strips so every core's units share one
   column tensor; the per-(core,unit) pair identity lives in UNIT_PAIRS and
   is applied on the host in combine(). Every cell of each matrix is covered
   exactly once across the 8 cores.

RBF assembly on device: arg_ij = a_i.b_j/32768 - |a_i|^2/65536 - |b_j|^2/65536.
 - a_i.b_j/32768 : PE matmul with the strip lhsT pre-scaled by 2^-15 (exact).
 - -|b_j|^2/65536: a K=1 accumulating matmul (ones outer-product row term).
 - -|a_i|^2/65536: per-partition bias of the ACT Exp instruction.
ACT's fused accum_out gives the per-partition running sums for free.

All RBF-path matmul operands are float32r (TF32-like): 1 PE cycle/column vs
fp32's 4, and measured max rel err 1.6e-4 on HW dot products -- far inside
this loss's tolerance. Operand tiles are rounded to f32r by their producer
(DVE/ACT copy), which the BIR verifier requires.
"""

import numpy as np

N = 4096
D = 128
NCORES = 8
ROWS = N // NCORES            # 512 batch rows per core (MSE/KLD)
IMG_F = 3 * 64 * 64           # 12288
P = 128
T_ROW = ROWS // P             # 4 row tiles per strip
MSE_CHUNK = 2048
MSE_NCH = IMG_F // MSE_CHUNK  # 6
NSTRIP = 6                    # a-side strips per core
BCOLS = 2048                  # b-side column-half length
JG = 1024                     # psum group width for the rbf matmuls
NJG = BCOLS // JG             # 2 j-groups per strip-unit
Z_VAR = 2.0
SIGMA = 2.0 * D * Z_VAR       # 512
INV_2S = 1.0 / (D * SIGMA / 2.0)   # 1/32768 (exact power of two)
INV_S = 1.0 / (D * SIGMA)          # 1/65536

# Per-core MMD work assignment (see module docstring). Strip s of tensor T is
# rows [s*512, (s+1)*512); column-halves are rows [0:2048] / [2048:4096] used
# on the b side. "x" units are cross (pz,z) cells; with the b-half fixed per
# core, cross strips come from the OTHER tensor.
#   core 0: a = pz strips 0-5,            b = pz[0:2048]   units: pp x6
#   core 1: a = pz strips 6,7 + z 0-3,    b = pz[0:2048]   units: pp,pp,x,x,x,x
#   core 2: a = pz strips 0-5,            b = pz[2048:]    units: pp x6
#   core 3: a = pz strips 6,7 + z 4-7,    b = pz[2048:]    units: pp,pp,x,x,x,x
#   core 4: a = z strips 0-5,             b = z[0:2048]    units: zz x6
#   core 5: a = z strips 6,7 + pz 4-7,    b = z[0:2048]    units: zz,zz,x,x,x,x
#   core 6: a = z strips 0-5,             b = z[2048:]     units: zz x6
#   core 7: a = z strips 6,7 + pz 0-3,    b = z[2048:]     units: zz,zz,x,x,x,x
UNIT_PAIRS = [
    ["pp"] * 6,
    ["pp", "pp", "x", "x", "x", "x"],
    ["pp"] * 6,
    ["pp", "pp", "x", "x", "x", "x"],
    ["zz"] * 6,
    ["zz", "zz", "x", "x", "x", "x"],
    ["zz"] * 6,
    ["zz", "zz", "x", "x", "x", "x"],
]

_CACHE = {}


def _build():
    import concourse.bass as bass
    import concourse.tile as tile
    from concourse import bacc, mybir

    f32 = mybir.dt.float32
    f32r = mybir.dt.float32r
    AF = mybir.ActivationFunctionType
    ALU = mybir.AluOpType
    AX = mybir.AxisListType

    nc = bacc.Bacc("TRN2", target_bir_lowering=False, debug=False,
                   num_devices=NCORES)

    r_blk = nc.dram_tensor("r_blk", [ROWS, IMG_F], f32, kind="ExternalInput").ap()
    x_blk = nc.dram_tensor("x_blk", [ROWS, IMG_F], f32, kind="ExternalInput").ap()
    a_nat = nc.dram_tensor("a_nat", [NSTRIP * ROWS, D], f32, kind="ExternalInput").ap()
    b_nat = nc.dram_tensor("b_nat", [BCOLS, D], f32, kind="ExternalInput").ap()
    mu_blk = nc.dram_tensor("mu_blk", [ROWS, D], f32, kind="ExternalInput").ap()
    lv_blk = nc.dram_tensor("lv_blk", [ROWS, D], f32, kind="ExternalInput").ap()

    NMSE = T_ROW * MSE_NCH            # 24 chunks
    NMSE_COLS = NMSE + 6              # last 2 chunks split into 4 strips each
    NMMD = NSTRIP * T_ROW * NJG       # 48 accum columns
    NTILE = NSTRIP * T_ROW            # 24 a-side row tiles
    mse_out = nc.dram_tensor("mse_acc", [P, NMSE_COLS], f32, kind="ExternalOutput").ap()
    mmd_out = nc.dram_tensor("mmd_acc", [P, NMMD], f32, kind="ExternalOutput").ap()
    kld_out = nc.dram_tensor("kld_acc", [P, 4], f32, kind="ExternalOutput").ap()

    with tile.TileContext(nc) as tc:
        with (
            tc.tile_pool(name="consts", bufs=1) as consts,
            tc.tile_pool(name="nat", bufs=1) as nat,
            tc.tile_pool(name="stream", bufs=4) as stream,
            tc.tile_pool(name="dpool", bufs=2) as dpool,
            tc.tile_pool(name="tstage", bufs=2) as tstage,
            tc.tile_pool(name="scratch", bufs=2) as scratch,
            tc.tile_pool(name="acc", bufs=1) as accp,
            tc.tile_pool(name="psmm", bufs=3, space="PSUM") as psmm,
            tc.tile_pool(name="pstr", bufs=2, space="PSUM") as pstr,
        ):
            # ---- constants / small setup ----
            # identity for PE transposes, built on the idle Pool engine:
            # ones tile, then keep only j == p (affine iota j - p == 0).
            ident_sb = consts.tile([P, P], f32)
            nc.gpsimd.memset(ident_sb[:], 1.0)
            nc.gpsimd.affine_select(out=ident_sb[:], in_=ident_sb[:],
                                    pattern=[[1, P]], base=0,
                                    channel_multiplier=-1,
                                    compare_op=ALU.is_equal, fill=0.0)
            ones_row_f = consts.tile([1, P], f32)
            nc.vector.memset(ones_row_f[:], 1.0)
            ones_row = consts.tile([1, P], f32r)   # memset can't emit f32r
            nc.scalar.activation(out=ones_row[:], in_=ones_row_f[:], func=AF.Copy)
            negs_col = consts.tile([P, 1], f32)    # -1/65536 column for norm matmuls
            nc.vector.memset(negs_col[:], -INV_S)

            # accumulators
            mse_cols = accp.tile([P, NMSE_COLS], f32)
            mmd_cols = accp.tile([P, NMMD], f32)
            kld_cols = accp.tile([P, 4], f32)
            nc.vector.memset(kld_cols[:, 3:4], 0.0)

            av = a_nat.rearrange("(t p) d -> p t d", p=P)   # 24 row tiles
            bv = b_nat.rearrange("(t p) d -> p t d", p=P)   # 16 row tiles

            rv = r_blk.rearrange("(t p) f -> p t f", p=P)
            xv = x_blk.rearrange("(t p) f -> p t f", p=P)

            def emit_mse(k):
                # MSE stream unit: DMA r/x chunk, DVE subtract, ACT square+accum.
                t, c = divmod(k, MSE_NCH)
                rt = stream.tile([P, MSE_CHUNK], f32, tag="rt")
                xt = stream.tile([P, MSE_CHUNK], f32, tag="xt")
                # 2-way splits keep >=16 transfers in flight (saturating the
                # 16 HW queues) at half the fixed ~0.6us-per-DMA SP-sequencer
                # issue cost; the last two chunks use finer (narrowing) splits
                # so their sub/square strips drain against the DMA stream.
                if k == NMSE - 1:
                    widths = [768, 768, 256, 256]
                elif k == NMSE - 2:
                    widths = [512] * 4
                else:
                    widths = [MSE_CHUNK // 2] * 2
                lo = c * MSE_CHUNK
                for w in widths:
                    o = lo - c * MSE_CHUNK
                    nc.sync.dma_start(out=rt[:, o:o + w],
                                      in_=rv[:, t, lo:lo + w])
                    nc.sync.dma_start(out=xt[:, o:o + w],
                                      in_=xv[:, t, lo:lo + w])
                    lo += w
                dt = dpool.tile([P, MSE_CHUNK], f32)
                sc = scratch.tile([P, MSE_CHUNK], f32, tag="msq")
                if k >= NMSE - 2:
                    # tail chunks: column strips aligned to the DMA splits,
                    # pipelining sub(DVE) against square(ACT). Extra accum
                    # columns: chunk NMSE-1 -> 24..26, chunk NMSE-2 -> 27..29.
                    base = NMSE + (NMSE - 1 - k) * 3
                    off = 0
                    for s, w in enumerate(widths):
                        col = k if s == 0 else base + s - 1
                        sl = slice(off, off + w)
                        off += w
                        nc.vector.tensor_sub(dt[:, sl], rt[:, sl], xt[:, sl])
                        nc.scalar.activation(out=sc[:, sl], in_=dt[:, sl],
                                             func=AF.Square,
                                             accum_out=mse_cols[:, col:col + 1])
                    return
                nc.vector.tensor_sub(dt[:], rt[:], xt[:])
                nc.scalar.activation(out=sc[:], in_=dt[:], func=AF.Square,
                                     accum_out=mse_cols[:, k:k + 1])

            # ---- transpose b half to [d, j] layout via PE (staged loads) ----
            # PSUM->SBUF copies (DVE, f32r-rounding) interleave with MSE
            # stream units so the DMA bus stays saturated from t=0.
            bT = consts.tile([P, BCOLS], f32r)
            mse_next = 0
            emit_mse(mse_next); mse_next += 1
            for g in range(2):                    # stage 8 row-tiles (1 MB)
                st = tstage.tile([P, 8, D], f32, tag="tst")
                nc.sync.dma_start(out=st[:], in_=bv[:, g * 8:g * 8 + 8, :])
                for gg in range(2):               # 4 transposes per psum tile
                    tp = pstr.tile([P, 512], f32, tag="tr")
                    for k in range(4):
                        nc.tensor.transpose(tp[:, k * P:(k + 1) * P],
                                            st[:, gg * 4 + k, :], ident_sb[:])
                    col = (g * 8 + gg * 4) * P
                    nc.vector.tensor_copy(bT[:, col:col + 512], tp[:])
                emit_mse(mse_next); mse_next += 1

            # ---- a strips natural + transposed/pre-scaled lhsT tiles ----
            a_t = nat.tile([P, NTILE, D], f32)
            nc.sync.dma_start(out=a_t[:, 0:12, :], in_=av[:, 0:12, :])
            nc.sync.dma_start(out=a_t[:, 12:24, :], in_=av[:, 12:24, :])
            aTs = consts.tile([P, NTILE * P], f32r)   # [d, i] per tile, *2^-15
            for h in range(NTILE // 4):
                tp = pstr.tile([P, 512], f32, tag="tr")
                for t in range(4):
                    nc.tensor.transpose(tp[:, t * P:(t + 1) * P],
                                        a_t[:, h * 4 + t, :], ident_sb[:])
                nc.scalar.activation(out=aTs[:, h * 512:(h + 1) * 512],
                                     in_=tp[:], func=AF.Copy, scale=INV_2S)
                if h % 3 == 2:
                    emit_mse(mse_next); mse_next += 1

            # ---- column norm row: negnorm[j] = -|b_j|^2/65536, laid [1, BCOLS] ----
            # squares on the (otherwise idle) Pool engine, norm matmul fp32,
            # PSUM->SBUF result copy on DVE with f32r rounding.
            nn_b = consts.tile([1, BCOLS], f32r)
            for c in range(BCOLS // 512):
                sq = scratch.tile([P, 512], f32, tag="sq")
                nc.gpsimd.tensor_mul(sq[:], bT[:, c * 512:(c + 1) * 512].bitcast(f32),
                                     bT[:, c * 512:(c + 1) * 512].bitcast(f32))
                npm = pstr.tile([P, 512], f32, tag="tr")
                nc.tensor.matmul(npm[0:1, :], lhsT=negs_col[:], rhs=sq[:],
                                 start=True, stop=True)
                nc.vector.tensor_copy(nn_b[0:1, c * 512:(c + 1) * 512],
                                      npm[0:1, :])
            emit_mse(mse_next); mse_next += 1

            # ---- row-bias tile: bias_a[:, ti] = -|a_i|^2/65536 per a row tile ----
            bias_a = consts.tile([P, NTILE], f32)
            for t in range(NTILE):
                sq2 = scratch.tile([P, D], f32, tag="sq2")
                # Square(x/256) = x^2/65536 (scale is an exact pow2)
                nc.scalar.activation(out=sq2[:], in_=a_t[:, t, :],
                                     func=AF.Square, scale=1.0 / 256.0,
                                     accum_out=bias_a[:, t:t + 1])
            nc.gpsimd.tensor_scalar_mul(bias_a[:], bias_a[:], -1.0)

            # ---- KLD block terms ----
            mu_t = nat.tile([P, T_ROW, D], f32)
            lv_t = nat.tile([P, T_ROW, D], f32)
            nc.sync.dma_start(out=mu_t[:], in_=mu_blk.rearrange("(t p) d -> p t d", p=P))
            nc.sync.dma_start(out=lv_t[:], in_=lv_blk.rearrange("(t p) d -> p t d", p=P))
            ksc = scratch.tile([P, T_ROW, D], f32, tag="ksc")
            nc.vector.tensor_reduce(kld_cols[:, 0:1], lv_t[:], axis=AX.XY,
                                    op=ALU.add)
            nc.scalar.activation(out=ksc[:], in_=mu_t[:], func=AF.Square,
                                 accum_out=kld_cols[:, 1:2])
            ksc2 = scratch.tile([P, T_ROW, D], f32, tag="ksc")
            nc.scalar.activation(out=ksc2[:], in_=lv_t[:], func=AF.Exp,
                                 accum_out=kld_cols[:, 2:3])

            # ---- interleaved main loops: MMD rbf blocks + MSE stream ----
            def emit_mmd(k):
                # k -> (a row tile ti, j-group g); 24 tiles x 2 groups
                ti, g = divmod(k, NJG)
                ps = psmm.tile([P, JG], f32)
                for jc in range(JG // 512):
                    j = g * (JG // 512) + jc
                    nc.tensor.matmul(ps[:, jc * 512:(jc + 1) * 512],
                                     lhsT=aTs[:, ti * P:(ti + 1) * P],
                                     rhs=bT[:, j * 512:(j + 1) * 512],
                                     start=True, stop=False)
                    nc.tensor.matmul(ps[:, jc * 512:(jc + 1) * 512],
                                     lhsT=ones_row[:], rhs=nn_b[0:1, j * 512:(j + 1) * 512],
                                     start=False, stop=True)
                sc = scratch.tile([P, JG], f32, tag="esc")
                nc.scalar.activation(out=sc[:], in_=ps[:], func=AF.Exp,
                                     bias=bias_a[:, ti:ti + 1], scale=1.0,
                                     accum_out=mmd_cols[:, k:k + 1])

            # Front-load MMD 3:1 against the MSE stream: all MMD (PE/ACT) work
            # finishes well before the DMA stream ends, so the kernel tail is
            # just the last chunk's subtract+square.
            mmd_next = 0
            while mmd_next < NMMD or mse_next < NMSE:
                for _ in range(3):
                    if mmd_next < NMMD:
                        emit_mmd(mmd_next)
                        mmd_next += 1
                if mse_next < NMSE:
                    emit_mse(mse_next)
                    mse_next += 1

            # ---- write partials out ----
            nc.sync.dma_start(out=mse_out, in_=mse_cols[:])
            nc.sync.dma_start(out=mmd_out, in_=mmd_cols[:])
            nc.sync.dma_start(out=kld_out, in_=kld_cols[:])

    nc.compile()
    return nc


def get_nc():
    if "nc" not in _CACHE:
        _CACHE["nc"] = _build()
    return _CACHE["nc"]


def make_in_maps(recons, x, z, mu, log_var, prior_z):
    r2 = np.ascontiguousarray(recons, dtype=np.float32).reshape(N, IMG_F)
    x2 = np.ascontiguousarray(x, dtype=np.float32).reshape(N, IMG_F)
    z = np.ascontiguousarray(z, dtype=np.float32)
    pz = np.ascontiguousarray(prior_z, dtype=np.float32)
    mu = np.ascontiguousarray(mu, dtype=np.float32)
    lv = np.ascontiguousarray(log_var, dtype=np.float32)

    S = ROWS  # 512-row strip
    H = BCOLS  # 2048-row column half
    # (a strips, b half) per core; see UNIT_PAIRS for the pair identities.
    a_list = [
        pz[0:6 * S],
        np.concatenate([pz[6 * S:8 * S], z[0:4 * S]]),
        pz[0:6 * S],
        np.concatenate([pz[6 * S:8 * S], z[4 * S:8 * S]]),
        z[0:6 * S],
        np.concatenate([z[6 * S:8 * S], pz[4 * S:8 * S]]),
        z[0:6 * S],
        np.concatenate([z[6 * S:8 * S], pz[0:4 * S]]),
    ]
    b_list = [pz[0:H], pz[0:H], pz[H:2 * H], pz[H:2 * H],
              z[0:H], z[0:H], z[H:2 * H], z[H:2 * H]]

    maps = []
    for c in range(NCORES):
        s = slice(c * ROWS, (c + 1) * ROWS)
        maps.append({
            "r_blk": r2[s], "x_blk": x2[s],
            "a_nat": np.ascontiguousarray(a_list[c]),
            "b_nat": np.ascontiguousarray(b_list[c]),
            "mu_blk": mu[s], "lv_blk": lv[s],
        })
    return maps


def combine(results):
    mse_sum = 0.0
    s_pp = s_zz = s_x = 0.0
    kld_total = 0.0
    cols_per_unit = T_ROW * NJG   # 8 accum columns per strip-unit
    for c, res in enumerate(results):
        mse_sum += np.float64(res["mse_acc"]).sum()
        m = np.float64(res["mmd_acc"])
        for u, pair in enumerate(UNIT_PAIRS[c]):
            su = m[:, u * cols_per_unit:(u + 1) * cols_per_unit].sum()
            if pair == "pp":
                s_pp += su
            elif pair == "zz":
                s_zz += su
            else:
                s_x += su
        k = np.float64(res["kld_acc"])
        kld_total += ROWS * D + k[:, 0].sum() - k[:, 1].sum() - k[:, 2].sum()

    recons_loss = mse_sum / (N * IMG_F)
    mmd = (s_pp + s_zz - 2.0 * s_x) / (float(N) * float(N))
    kld = -0.5 * kld_total / N
    beta, alpha, reg_w = 5.0, -0.5, 100.0
    loss = (beta * recons_loss
            + (1.0 - alpha) * (1.0 / N) * kld
            + (alpha + reg_w - 1.0) / (float(N) * (N - 1)) * mmd)
    return (np.float32(loss), np.float32(recons_loss),
            np.float32(mmd), np.float32(-kld))


def run(recons, x, z, mu, log_var, prior_z, trace=False):
    from concourse.bass_utils import run_bass_kernel_spmd
    nc = get_nc()
    in_maps = make_in_maps(recons, x, z, mu, log_var, prior_z)
    res = run_bass_kernel_spmd(nc, in_maps, list(range(NCORES)), trace=trace)
    return res


def kernel(recons, x, z, mu, log_var, prior_z):
    res = run(recons, x, z, mu, log_var, prior_z)
    return combine(res.results)
